# revision 4
# baseline (speedup 1.0000x reference)
"""Trainium2 Bass kernel v3 for relational GNN message passing (BlockDecomposition).

Strategy (8 NeuronCores, SPMD, no collectives):
  - Symmetrize edges into 1.6M directed messages; shard by destination node
    range (6250 nodes/core, 49 buckets of 128). Self-loop handled densely
    (no messages): out_ps += BW_self^T @ (keep-masked own-shard x^T tile).
  - x lives ENTIRELY in SBUF as a bf16 token table (token i at partition
    i % 128, 256B rank stripe i // 128). Per-message rows come from the
    SBUF-source dma_gather (transpose=True) -> GT [feat, msg] tiles.
    int16 idx limit 32768 keeps the lo/hi window split at row 32768.
  - Transform-first: per 128-msg tile, XW_ps = GT_tile^T @ BW_rel lands
    msg-major in PSUM (the transform doubles as the layout fix-up).
    Four tiles share one PSUM bank; one quad copy evacuates to SBUF bf16
    (alternating DVE/ACT).
  - Weighted one-hot Hw tile in ONE DVE tensor_scalar:
    Hw = (iota == grow_col) * gw_col  (per-partition scalars; pad slots
    grow=255 -> zero column, gw=0).
  - Aggregation: out_ps[feat', dst] += XW_sb^T @ Hw accumulated over all
    tiles of the bucket regardless of relation (transform already applied).
  - Host transposes/concatenates per-core outputs.
"""

import sys

import numpy as np

sys.path.insert(0, "/opt/trn_rl_repo")

N_NODES = 50000
DIM = 128
N_REL = 8  # edge relations; self-loop handled densely
NCORES = 8
SHARD = N_NODES // NCORES  # 6250
NBUCK = (SHARD + 127) // 128  # 49
PADN = NBUCK * 128  # 6272
LO = 32768  # int16-addressable tokens per gather window
NRANKS = (N_NODES + 127) // 128  # 391

_PAD_ROW = 255.0

_cache = {}
last_result = None


def _np_dt(dt_name):
    if dt_name == "float32":
        return np.float32
    import ml_dtypes

    return np.dtype(getattr(ml_dtypes, dt_name))


class Layout:
    def __init__(self, tlo, thi):
        self.tlo = tlo  # [NBUCK, N_REL]
        self.thi = thi
        self.nlo = tlo.sum(axis=1)
        self.nhi = thi.sum(axis=1)
        self.tb = self.nlo + self.nhi
        self.c0 = np.zeros(NBUCK, dtype=np.int64)
        self.c0[1:] = np.cumsum(self.tb)[:-1]
        self.nt = int(self.tb.sum())
        self.lo_off = np.zeros((NBUCK, N_REL), dtype=np.int64)
        self.lo_off[:, 1:] = np.cumsum(tlo, axis=1)[:, :-1]
        self.hi_off = np.zeros((NBUCK, N_REL), dtype=np.int64)
        self.hi_off[:, 1:] = np.cumsum(thi, axis=1)[:, :-1]
        self.olo = np.zeros(NBUCK, dtype=np.int64)
        self.olo[1:] = np.cumsum(self.nlo * 8)[:-1]
        self.ohi = np.zeros(NBUCK, dtype=np.int64)
        self.ohi[1:] = np.cumsum(self.nhi * 8)[:-1]
        self.ilo_cols = int((self.nlo * 8).sum())
        self.ihi_cols = int((self.nhi * 8).sum())

    def key(self):
        return (self.tlo.tobytes(), self.thi.tobytes())


def _message_arrays(src, dst, rel, w, k):
    m = (dst >= k * SHARD) & (dst < (k + 1) * SHARD)
    return src[m], dst[m] - k * SHARD, rel[m], w[m]


def _prepare_layout(src, dst, rel, w):
    cnt = np.zeros((NCORES, NBUCK, N_REL, 2), dtype=np.int64)
    percore = []
    for k in range(NCORES):
        s_k, l_k, r_k, w_k = _message_arrays(src, dst, rel, w, k)
        half = (s_k >= LO).astype(np.int64)
        bucket = l_k // 128
        np.add.at(cnt[k], (bucket, r_k, half), 1)
        percore.append((s_k, l_k, r_k, w_k, half, bucket))
    tlo = -(-cnt[:, :, :, 0].max(axis=0) // 128)
    thi = -(-cnt[:, :, :, 1].max(axis=0) // 128)
    return Layout(tlo, thi), percore


def _prepare_core_meta(lay, percore, dt_name):
    npdt = _np_dt(dt_name)
    ilo_all, ihi_all, grow_all, gw_all = [], [], [], []
    for k in range(NCORES):
        s_k, l_k, r_k, w_k, half, bucket = percore[k]
        row = l_k % 128
        order = np.lexsort((s_k, half, r_k, bucket))
        s_k, r_k, w_k, half, bucket, row = (
            a[order] for a in (s_k, r_k, w_k, half, bucket, row)
        )
        g = (bucket * N_REL + r_k) * 2 + half
        sizes = np.bincount(g, minlength=NBUCK * N_REL * 2)
        starts = np.zeros_like(sizes)
        starts[1:] = np.cumsum(sizes)[:-1]
        rank = np.arange(len(g)) - starts[g]

        t_lo = lay.c0[bucket] + lay.lo_off[bucket, r_k] + rank // 128
        t_hi = lay.c0[bucket] + lay.nlo[bucket] + lay.hi_off[bucket, r_k] + rank // 128
        t = np.where(half == 0, t_lo, t_hi)
        p = rank % 128

        grow = np.full((128, lay.nt), _PAD_ROW, dtype=np.float32)
        gw = np.zeros((128, lay.nt), dtype=np.float32)
        grow[p, t] = row
        gw[p, t] = w_k

        t_in_region = np.where(
            half == 0, t - lay.c0[bucket], t - lay.c0[bucket] - lay.nlo[bucket]
        )
        i_pos = t_in_region * 128 + p
        col_off = np.where(half == 0, lay.olo[bucket], lay.ohi[bucket])
        cols = col_off + i_pos // 16
        prow = i_pos % 16
        ilo = np.zeros((16, lay.ilo_cols), dtype=np.int16)
        ihi = np.zeros((16, lay.ihi_cols), dtype=np.int16)
        is_lo = half == 0
        ilo[prow[is_lo], cols[is_lo]] = s_k[is_lo].astype(np.int16)
        ihi[prow[~is_lo], cols[~is_lo]] = (s_k[~is_lo] - LO).astype(np.int16)

        ilo_all.append(np.tile(ilo, (8, 1)))
        ihi_all.append(np.tile(ihi, (8, 1)))
        grow_all.append(grow)
        gw_all.append(gw)
    return ilo_all, ihi_all, grow_all, gw_all


def _build_program(dt_name, lay, repeat=1, nq=2):
    from contextlib import ExitStack

    from concourse import bacc, mybir
    import concourse.tile as tile

    DT = getattr(mybir.dt, dt_name)
    f32 = mybir.dt.float32
    i16 = mybir.dt.int16

    nc = bacc.Bacc(None, target_bir_lowering=False, debug=False, num_swdge_queues=nq)

    with tile.TileContext(nc) as tc:
        with tc.tile_pool(name="dram", bufs=1, space="DRAM") as dram:
            xsb_d = dram.tile([128, NRANKS * DIM], DT, kind="ExternalInput", name="xsb")
            xkt_d = dram.tile([128, PADN], DT, kind="ExternalInput", name="xkt")
            bw_d = dram.tile([128, (N_REL + 1) * 128], DT, kind="ExternalInput", name="bw")
            iota_d = dram.tile([128, 128], DT, kind="ExternalInput", name="iota")
            ilo_d = dram.tile([128, lay.ilo_cols], i16, kind="ExternalInput", name="ilo")
            ihi_d = dram.tile([128, lay.ihi_cols], i16, kind="ExternalInput", name="ihi")
            grow_d = dram.tile([128, lay.nt], f32, kind="ExternalInput", name="grow")
            gw_d = dram.tile([128, lay.nt], f32, kind="ExternalInput", name="gw")
            out_d = dram.tile([128, PADN], f32, kind="ExternalOutput", name="outT")

            max_tb = int(lay.tb.max())
            with (
                tc.tile_pool(name="const", bufs=1) as constp,
                tc.tile_pool(name="gpool", bufs=2) as gpool,
                tc.tile_pool(name="hpool", bufs=8) as hpool,
                tc.tile_pool(name="xwsb", bufs=4) as xwsbp,
                tc.tile_pool(name="outsb", bufs=3) as outsbp,
                tc.tile_pool(name="xwps", bufs=3, space="PSUM") as xwpsp,
                tc.tile_pool(name="outps", bufs=2, space="PSUM") as outpsp,
            ):
                iota_s = constp.tile([128, 128], DT)
                bw_s = constp.tile([128, (N_REL + 1) * 128], DT)
                grow_s = constp.tile([128, lay.nt], f32)
                gw_s = constp.tile([128, lay.nt], f32)
                xkt_s = constp.tile([128, PADN], DT)
                ilo_s = constp.tile([128, lay.ilo_cols], i16)
                ihi_s = constp.tile([128, lay.ihi_cols], i16)
                x_s = constp.tile([128, NRANKS * DIM], DT)
                nc.sync.dma_start(out=iota_s[:], in_=iota_d[:])
                nc.sync.dma_start(out=bw_s[:], in_=bw_d[:])
                nc.sync.dma_start(out=grow_s[:], in_=grow_d[:])
                nc.sync.dma_start(out=gw_s[:], in_=gw_d[:])
                nc.sync.dma_start(out=xkt_s[:], in_=xkt_d[:])
                nc.sync.dma_start(out=ilo_s[:], in_=ilo_d[:])
                nc.sync.dma_start(out=ihi_s[:], in_=ihi_d[:])
                nc.sync.dma_start(out=x_s[:], in_=xsb_d[:])

                qi = 0
                copy_i = 0
                rep_ctx = ExitStack()
                if repeat > 1:
                    rep_ctx.enter_context(tc.For_i(0, repeat, 1))
                for b in range(NBUCK):
                    c0 = int(lay.c0[b])
                    nlo = int(lay.nlo[b])
                    nhi = int(lay.nhi[b])
                    tb = nlo + nhi
                    GT = gpool.tile([128, 1, max_tb * 128], DT, name="GT")
                    nc.gpsimd.dma_gather(
                        GT[:, :, 0 : nlo * 128],
                        x_s[:, 0 : 256 * DIM],
                        ilo_s[:, int(lay.olo[b]) : int(lay.olo[b]) + 8 * nlo],
                        nlo * 128,
                        nlo * 128,
                        DIM,
                        transpose=True,
                        single_packet=False,
                        sbuf_tokens_per_rank=128,
                        sbuf_free_dim_per_rank=256,
                        sbuf_free_dim_pad_per_rank=0,
                        sbuf_byte_offset=0,
                        queue_num=qi % nq,
                    )
                    qi += 1
                    nc.gpsimd.dma_gather(
                        GT[:, :, nlo * 128 : tb * 128],
                        x_s[:, 256 * DIM : NRANKS * DIM],
                        ihi_s[:, int(lay.ohi[b]) : int(lay.ohi[b]) + 8 * nhi],
                        nhi * 128,
                        nhi * 128,
                        DIM,
                        transpose=True,
                        single_packet=False,
                        sbuf_tokens_per_rank=128,
                        sbuf_free_dim_per_rank=256,
                        sbuf_free_dim_pad_per_rank=0,
                        sbuf_byte_offset=0,
                        queue_num=qi % nq,
                    )
                    qi += 1

                    # tile t (within bucket) -> relation of that tile
                    rels = []
                    for r in range(N_REL):
                        rels += [r] * int(lay.tlo[b, r])
                    for r in range(N_REL):
                        rels += [r] * int(lay.thi[b, r])

                    out_ps = outpsp.tile([128, 128], f32, name="out_ps", space="PSUM")
                    # dense self-loop: BW_self^T @ (keep * x_own)^T
                    nc.tensor.matmul(
                        out=out_ps[:],
                        lhsT=bw_s[:, N_REL * 128 : (N_REL + 1) * 128],
                        rhs=xkt_s[:, b * 128 : (b + 1) * 128],
                        start=True,
                        stop=(tb == 0),
                    )

                    nquad = (tb + 3) // 4
                    for q in range(nquad):
                        lo_t = q * 4
                        hi_t = min(lo_t + 4, tb)
                        nk = hi_t - lo_t
                        xw_ps = xwpsp.tile([128, 512], f32, name="xw_ps", space="PSUM")
                        for j in range(lo_t, hi_t):
                            nc.tensor.matmul(
                                out=xw_ps[:, (j - lo_t) * 128 : (j - lo_t + 1) * 128],
                                lhsT=GT[:, 0, j * 128 : (j + 1) * 128],
                                rhs=bw_s[:, rels[j] * 128 : (rels[j] + 1) * 128],
                                start=True,
                                stop=True,
                            )
                        xw_sb = xwsbp.tile([128, 512], DT, name="xw_sb")
                        if copy_i % 2 == 0:
                            nc.scalar.copy(out=xw_sb[:, : nk * 128], in_=xw_ps[:, : nk * 128])
                        else:
                            nc.vector.tensor_copy(
                                out=xw_sb[:, : nk * 128], in_=xw_ps[:, : nk * 128]
                            )
                        copy_i += 1
                        for j in range(lo_t, hi_t):
                            H = hpool.tile([128, 128], DT, name="H", tag="H")
                            nc.vector.tensor_scalar(
                                out=H[:],
                                in0=iota_s[:],
                                scalar1=grow_s[:, c0 + j : c0 + j + 1],
                                scalar2=gw_s[:, c0 + j : c0 + j + 1],
                                op0=mybir.AluOpType.is_equal,
                                op1=mybir.AluOpType.mult,
                            )
                            nc.tensor.matmul(
                                out=out_ps[:],
                                lhsT=xw_sb[:, (j - lo_t) * 128 : (j - lo_t + 1) * 128],
                                rhs=H[:],
                                start=False,
                                stop=(j == tb - 1),
                            )
                    out_sb = outsbp.tile([128, 128], f32, name="out_sb")
                    nc.scalar.copy(out=out_sb[:], in_=out_ps[:])
                    nc.sync.dma_start(out=out_d[:, b * 128 : (b + 1) * 128], in_=out_sb[:])
                rep_ctx.close()

    nc.compile()
    names = {
        "xsb": xsb_d.tensor.name,
        "xkt": xkt_d.tensor.name,
        "bw": bw_d.tensor.name,
        "iota": iota_d.tensor.name,
        "ilo": ilo_d.tensor.name,
        "ihi": ihi_d.tensor.name,
        "grow": grow_d.tensor.name,
        "gw": gw_d.tensor.name,
        "out": out_d.tensor.name,
    }
    return nc, names


def _block_diag_bw(blocks, dt_name):
    npdt = _np_dt(dt_name)
    nrel1, nb, bs, _ = blocks.shape
    bw = np.zeros((128, nrel1 * 128), dtype=np.float32)
    for r in range(nrel1):
        for a in range(nb):
            bw[a * bs : (a + 1) * bs, r * 128 + a * bs : r * 128 + (a + 1) * bs] = blocks[r, a]
    return bw.astype(npdt)


def _prep(x, blocks, node_keep_mask, source, target, edge_type, edge_weights, _dt):
    x = np.asarray(x, dtype=np.float32)
    blocks = np.asarray(blocks, dtype=np.float32)
    keep = np.asarray(node_keep_mask).astype(bool)
    source = np.asarray(source).astype(np.int64)
    target = np.asarray(target).astype(np.int64)
    edge_type = np.asarray(edge_type).astype(np.int64)
    edge_weights = np.asarray(edge_weights, dtype=np.float32)

    npdt = _np_dt(_dt)
    src = np.concatenate([source, target])
    dst = np.concatenate([target, source])
    rel = np.concatenate([edge_type, edge_type])
    w = np.concatenate([edge_weights, edge_weights])

    lay, percore = _prepare_layout(src, dst, rel, w)
    ilo_all, ihi_all, grow_all, gw_all = _prepare_core_meta(lay, percore, _dt)

    xt = x.astype(npdt)
    xpad = np.zeros((NRANKS * 128, DIM), dtype=npdt)
    xpad[:N_NODES] = xt
    xsb = np.ascontiguousarray(
        xpad.reshape(NRANKS, 128, DIM).transpose(1, 0, 2).reshape(128, NRANKS * DIM)
    )
    xk = x * keep[:, None].astype(np.float32)
    xkt_all = []
    for k in range(NCORES):
        xkt = np.zeros((128, PADN), dtype=np.float32)
        xkt[:, :SHARD] = xk[k * SHARD : (k + 1) * SHARD].T
        xkt_all.append(xkt.astype(npdt))

    bw = _block_diag_bw(blocks, _dt)
    iota = np.tile(np.arange(128, dtype=np.float32), (128, 1)).astype(npdt)
    data = (xsb, xkt_all, bw, iota, ilo_all, ihi_all, grow_all, gw_all)
    return lay, data


def _in_maps(names, data):
    xsb, xkt_all, bw, iota, ilo_all, ihi_all, grow_all, gw_all = data
    return [
        {
            names["xsb"]: xsb,
            names["xkt"]: xkt_all[k],
            names["bw"]: bw,
            names["iota"]: iota,
            names["ilo"]: ilo_all[k],
            names["ihi"]: ihi_all[k],
            names["grow"]: grow_all[k],
            names["gw"]: gw_all[k],
        }
        for k in range(NCORES)
    ]


def _get_program(_dt, lay, repeat=1):
    key = (_dt,) + lay.key() + (repeat,)
    if key not in _cache:
        _cache[key] = _build_program(_dt, lay, repeat)
    return _cache[key]


def kernel(x, blocks, node_keep_mask, source, target, edge_type, edge_weights, _dt="bfloat16"):
    from concourse.bass_utils import run_bass_kernel_spmd

    lay, data = _prep(
        x, blocks, node_keep_mask, source, target, edge_type, edge_weights, _dt
    )
    nc, names = _get_program(_dt, lay)
    res = run_bass_kernel_spmd(nc, _in_maps(names, data), list(range(NCORES)))
    global last_result
    last_result = res
    out = np.concatenate(
        [np.asarray(res.results[k][names["out"]]).T[:SHARD] for k in range(NCORES)],
        axis=0,
    ).astype(np.float32)
    return out


def measure_hw_ns(inputs, _dt="bfloat16", big_rep=1025, n_runs=4):
    import time

    from concourse.bass_utils import run_bass_kernel_spmd

    lay, data = _prep(_dt=_dt, **inputs)
    walls = {}
    out_big = None
    for rep in (1, big_rep):
        nc, names = _get_program(_dt, lay, rep)
        maps = _in_maps(names, data)
        res = run_bass_kernel_spmd(nc, maps, list(range(NCORES)))
        best = float("inf")
        for _ in range(n_runs):
            t0 = time.perf_counter()
            res = run_bass_kernel_spmd(nc, maps, list(range(NCORES)))
            best = min(best, time.perf_counter() - t0)
        walls[rep] = best
        if rep == big_rep:
            out_big = np.concatenate(
                [
                    np.asarray(res.results[k][names["out"]]).T[:SHARD]
                    for k in range(NCORES)
                ],
                axis=0,
            ).astype(np.float32)
    body_ns = (walls[big_rep] - walls[1]) / (big_rep - 1) * 1e9
    print(
        f"wall rep=1: {walls[1] * 1e3:.1f} ms, rep={big_rep}: "
        f"{walls[big_rep] * 1e3:.1f} ms -> body {body_ns:.0f} ns"
    )
    return body_ns, out_big
